# revision 25
# baseline (speedup 1.0000x reference)
"""AdaptiveGaussianConvLayer Trainium2 kernel (8 NeuronCores, SPMD, no collectives).

Math: out[b, j, d] = sum_i V[b, i, d] * W[b, i, j],
      W[b, i, j] = exp(-0.5 * ((j - i - mu[b,i]) / sigma[b,i])^2)
with B=4, N=4096, D=512; sigma in (0.5, 2.5), mu ~ 3*N(0,1).

W underflows to exactly 0.0 once |j - i - mu|/sigma >= ~13.2, i.e. for
|j - i| >= ~48.  On a 64-shifted slab grid (slab s = rows [128s - 64,
128s + 64) of the core's j-range), each 128-wide j-tile t needs only slabs
{t, t+1}, covering i in [128t - 64, 128t + 192) — a superset of the true
band — so the banded result matches the dense reference to rounding.

Sharding: 8 cores = (batch b) x (j-half h).  Core c computes
out[b, h*2048:(h+1)*2048, :].  Host pads V/sigma/mu with 64 zero rows on
each side of the core's i-window so all cores run one identical SPMD
program.  V is pre-cast to bf16 on the host (scaled by sqrt(pi)/2, see
below); the OUTPUT is written as bf16 and widened to fp32 on the host
(halves the dominant output DMA; adds ~1e-3 rel err, inside the 2e-2 gate).

W is produced via the identity
    exp(-0.5 z^2) = sqrt(pi)/2 * d/dx erf(x) |_{x = z/sqrt(2)}
with the sqrt(pi)/2 constant folded into V on the host; the Derivative_Erf
pwp table saturates to 0 for |x| >= ~9.87 (z >= ~13.96), which keeps the
+-48 band exact.  This replaces the whole Square+Exp pipeline with one
table lookup: slabs 0-1 are single fused ops on ACT (Derivative_Erf of
rh*iota + b0h, so the matmul chain starts as soon as the tiny cst DMA
lands); for the rest, DVE and GpSimd alternate the affine step
z'_s = (iota + q_s) * rh_s (tensor_scalar; iota and z' are bf16 so the
DVE runs it in 2x_1P mode at ~0.34us/slab) and ACT sweeps Derivative_Erf
over multi-slab chunks at (N+352)/1.2 ns.

Per-core dataflow (i on partitions, j/d on the free axis):
  z'_s  = (iota + q_s) * rh_s                      (DVE/GpSimd, fp32)
  W     = Derivative_Erf(z') in multi-slab chunks  (ACT, bf16 out)
  psum  = sum_{k=0,1} W[slab t+k].T @ V[slab t+k]  (TensorE, K=128 bf16)
  obuf  <- [128,1024] psum pair-copy w/ bf16 cast (DVE early pairs, ACT
           Copy late ones — Copy shares the erf_derivative table set, so
           there is exactly one ACT_TABLE_LOAD), DMA out per pair.

The steady state is copy-bound: PSUM is readable only by DVE and ACT
(GpSimd has no PSUM port), so the psum->SBUF cast-copies set the tile
cadence alongside the ~360 GB/s DMA system.  Copies run at PAIR
granularity — each [128,1024] copy drains a 2-bank psum pair in one
instruction, cutting per-op overhead and semaphore count.  cst goes
first on the SP HWDGE ring (its descriptors beat the V flood to the DMA
engines; everything hangs off it), then V groups, then output pairs in
data-ready order — one ring fans out across all 16 DMA engines.  Both V
and the output use partition-major DRAM layouts (host-side repack) so
every DMA descriptor is a 2KB+ contiguous run.

Tail: the NRT-injected NEFF epilogue is [all-engine barrier; per-sem
clears of S[3..255] split 51/engine; barrier; NOTIFY] — the barrier means
the ~6 us clear chain cannot be overlapped, but we still drop our own
end-of-program barriers (the runtime's suffices), keep SP as the only
engine with terminal DMA waits, and hand GpSimd its DGE-queue reset via a
single done-sem hop.  All live sems are pushed into SP's clear partition
(207+) by padding the allocator, so early-exiting engines' clear chains
never touch a live semaphore.

A handful of scratch matmuls (gated only on local memsets, so they start
within ~0.3 us) keep the PE HAM activity window busy from t=0 so the
clock gate releases (1.2 -> 2.4 GHz) as early as its free-running 3.4 us
window allows.
"""

import os
import math
import numpy as np
import ml_dtypes

import concourse.bass as bass
import concourse.bacc as bacc
import concourse.mybir as mybir
import concourse.tile as tile
from concourse.bass_utils import run_bass_kernel_spmd
from concourse.vector_clock import ScopedClock

AF = mybir.ActivationFunctionType
ALU = mybir.AluOpType

B, N, D = 4, 4096, 512
NCORES = 8
HALF = N // 2             # 2048 j per core
NSLAB = HALF // 128 + 1   # 17 slabs of 128 rows on the 64-shifted grid
VROWS = NSLAB * 128       # 2176
JT = HALF // 128          # 16 j-tiles per core
WWIN = 256                # j-window width per slab

SQRT_PI_2 = math.sqrt(math.pi) / 2.0     # folded into V on the host
INV_SQRT2 = 1.0 / math.sqrt(2.0)

# genuinely used j-window per slab (edge slabs serve one j-tile)
def _slab_win(s):
    t_lo, t_hi = max(s - 1, 0), min(s, JT - 1)
    lo = (t_lo - (s - 1)) * 128
    return lo, (t_hi - t_lo + 1) * 128

# psum->obuf copy engine per j-tile PAIR ('d'=DVE CAST, 'a'=ACT Copy);
# one [128,1024] copy drains a 2-bank psum pair in a single instruction
COPY_ENG = os.environ.get("AGC_COPY", "ddddaaaa")

# slabs 0,1 are produced by a direct per-slab D_ERF on ACT (no z'
# dependency, so the matmul chain starts ~2.5us in); the rest get a z'
# tensor_scalar, alternating DVE/GpSimd so the combined pace is ~0.3us/slab
ACT_DIRECT = (0, 1)
DVE_TS = tuple(int(x) for x in os.environ.get("AGC_DVE_TS", "2,3,4,6,8,10,12,14,16").split(",") if x != "")
GPS_TS = tuple(int(x) for x in os.environ.get("AGC_GPS_TS", "5,7,9,11,13,15").split(",") if x != "")

# Derivative_Erf chunk list (slab start, nslabs); j-tile t unlocks once
# slab t+1 is done
CHUNKS = [(2, 2), (4, 3), (7, 3), (10, 3), (13, 4)]

# V load slab groups (ring order = arrival order)
VGROUPS = ((0, 2), (2, 4), (4, 6), (6, 9), (9, 13), (13, 17))

WARMUP = int(os.environ.get("AGC_WARMUP", "8"))
TAILMM = int(os.environ.get("AGC_TAILMM", "0"))
FLATBAR = os.environ.get("AGC_FLATBAR", "1") == "1"

_cached = {}


def _flat_start_barrier(self, *, sem_only=False):
    """Flat all-engine barrier: every engine incs one sem and waits for the
    full count — one cross-engine hop instead of the stock sequential chain."""
    arrive = self.alloc_semaphore("flat_barrier_arrive")
    n = len(self.engines)
    for eng in self.engines.values():
        eng.sem_inc(arrive, 1)
    for eng in self.engines.values():
        eng.wait_ge(arrive, n)


def _minimal_tail(self, tick_clock, wait_clock):
    """Replace the stock drain + 2x all-engine barrier + range-clear tail.

    SP alone carries the terminal waits (DMA-completion + engine-progress
    sems — all allocated >= 207 so only SP's runtime clear partition
    touches them).  Every other engine falls straight off its stream into
    the NRT-injected epilogue.  GpSimd still resets the DGE queues for
    re-execution; it waits on one done-sem from SP (the sem lives in SP's
    partition, so SP's own clear chain re-zeroes it afterwards)."""
    nc = self.nc
    drain_inst = nc.sync.drain()
    wait_clock.add_sem_waits(
        drain_inst.ins, ScopedClock({None: tick_clock.global_clock})
    )
    popped = nc._tile_sem_poison_stack.pop()
    assert popped is self._sem_poison
    nc.sync.sem_inc(nc._agc_done_sem, 1)
    nc.gpsimd.wait_ge(nc._agc_done_sem, 1)
    nc.gpsimd.dma_reset()


def build_nc():
    tile.TileContext._drain_and_barrier = _minimal_tail
    f32 = mybir.dt.float32
    bf16 = mybir.dt.bfloat16
    orig_barrier = bass.Bass.all_engine_barrier
    if FLATBAR:
        bass.Bass.all_engine_barrier = _flat_start_barrier
    try:
        nc = bacc.Bacc("TRN2", target_bir_lowering=False, debug=False)
    finally:
        bass.Bass.all_engine_barrier = orig_barrier

    # Pad the allocator so everything the kernel actually uses (done-sem,
    # flat-barrier sem, engine sems, HWDGE sems) lands in SP's runtime
    # clear partition 207+.
    _pad_i = 0
    while True:
        s = nc.alloc_semaphore(f"agc_pad_{_pad_i}")
        _pad_i += 1
        if s.num >= 206:
            break
    nc._agc_done_sem = nc.alloc_semaphore("agc_done")
    assert nc._agc_done_sem.num >= 207, nc._agc_done_sem.num

    # V pre-cast to bf16 (scaled by sqrt(pi)/2) AND pre-tiled
    # partition-major on the host: Vp[p, s*D+d] = V[row 128s+p, d]
    vp_d = nc.dram_tensor("Vp", [128, NSLAB * D], bf16, kind="ExternalInput").ap()
    # cst = [iota(256) | q rh pairs | b0h rh pairs | zero] per partition
    CW = WWIN + 4 * NSLAB + 1
    cst_d = nc.dram_tensor("cst", [128, CW], f32, kind="ExternalInput").ap()
    # bf16 iota: with bf16 in AND out the DVE tensor_scalar runs in 2x_1P
    # mode (2 elem/cycle), halving the z' chain on the critical path
    cst2_d = nc.dram_tensor("cst2", [128, WWIN], bf16, kind="ExternalInput").ap()
    # partition-major output: out_d[p, t*D+d] = out[row 128t+p, d] — every
    # partition's data is one contiguous 16KB run, so out-DMA descriptors
    # are 2KB (full per-engine DMA bandwidth); the host de-interleaves
    out_d = nc.dram_tensor("out", [128, JT * D], bf16, kind="ExternalOutput").ap()

    with tile.TileContext(nc) as tc:
        with (
            tc.tile_pool(name="const", bufs=1) as constp,
            tc.tile_pool(name="big", bufs=1) as bigp,
            tc.tile_pool(name="ps", bufs=4, space=bass.MemorySpace.PSUM) as pspool,
            tc.tile_pool(name="obuf", bufs=8) as opool,
        ):
            cst_t = constp.tile([128, CW], f32, name="cst_t")
            cst2_t = constp.tile([128, WWIN], bf16, name="cst2_t")
            vbuf = bigp.tile([128, NSLAB * D], bf16, name="vbuf")
            wbuf = bigp.tile([128, NSLAB * WWIN], bf16, name="wbuf")
            zbuf = bigp.tile([128, NSLAB * WWIN], bf16, name="zbuf")

            # cst goes FIRST on the SP ring so its descriptors reach the
            # DMA engines ahead of the V flood (everything the W chain
            # depends on hangs off this one small transfer)
            nc.sync.dma_start(cst_t[:], cst_d[:])
            nc.sync.dma_start(cst2_t[:], cst2_d[:])
            for lo, hi in VGROUPS:
                nc.sync.dma_start(vbuf[:, lo * D : hi * D], vp_d[:, lo * D : hi * D])

            iota_t = cst_t[:, 0:WWIN]
            br = lambda s: (cst_t[:, WWIN + 2 * s : WWIN + 2 * s + 1],
                            cst_t[:, WWIN + 2 * s + 1 : WWIN + 2 * s + 2])

            # PE warm-up: scratch matmuls gated only on local memsets so the
            # HAM activity window sees a busy PE from ~t=0
            wscr = bigp.tile([128, 128], bf16, name="wscr")
            nc.gpsimd.memset(wscr[:], 0.0)
            wscr2 = bigp.tile([128, D], bf16, name="wscr2")
            nc.gpsimd.memset(wscr2[:], 0.0)
            # zero the never-written z' edge gaps read by chunked D_ERF
            nc.gpsimd.memset(zbuf[:, 0:128], 0.0)
            nc.gpsimd.memset(zbuf[:, 16 * WWIN + 128 : 17 * WWIN], 0.0)
            # warm-up matmuls land in pair 0's first half; tile 0's real
            # matmul overwrites them (start=True)
            ps0 = pspool.tile([128, 2 * D], f32, tag="ps", name="ps0")
            for _ in range(WARMUP):
                nc.tensor.matmul(ps0[:, 0:D], wscr[:], wscr2[:],
                                 start=True, stop=True)

            # z' slab: z'_s = (iota + q_s) * rh_s in fp32 (DVE runs this at
            # 2 elem/cycle — single-src fp32 SBUF tensor_scalar; GpSimd
            # takes the late slabs); bias/scale table holds (q, rh) pairs
            def emit_ts(s):
                lo, w = _slab_win(s)
                dst = zbuf[:, s * WWIN + lo : s * WWIN + lo + w]
                src = cst2_t[:, lo : lo + w]
                q, rh = br(s)
                eng = nc.vector if s in DVE_TS else nc.gpsimd
                eng.tensor_scalar(dst, src, q, rh, ALU.add, ALU.mult)

            # direct per-slab Gaussian on ACT: D_ERF(rh*iota + b0h)
            bh = lambda s: (cst_t[:, WWIN + 2 * NSLAB + 2 * s : WWIN + 2 * NSLAB + 2 * s + 1],
                            cst_t[:, WWIN + 2 * NSLAB + 2 * s + 1 : WWIN + 2 * NSLAB + 2 * s + 2])

            def emit_w_direct(s):
                lo, w = _slab_win(s)
                dst = wbuf[:, s * WWIN + lo : s * WWIN + lo + w]
                src = iota_t[:, lo : lo + w]
                b0h, rh = bh(s)
                nc.scalar.activation(dst, src, AF.Derivative_Erf,
                                     bias=b0h, scale=rh)

            # W chunk: Gaussian via scaled derivative-of-erf over z'
            zbias = cst_t[:, CW - 1 : CW]    # column of zeros

            def emit_w_chunk(ci):
                s0, ns = CHUNKS[ci]
                lo = s0 * WWIN + _slab_win(s0)[0]
                last = s0 + ns - 1
                hi = last * WWIN + sum(_slab_win(last))
                nc.scalar.activation(wbuf[:, lo:hi], zbuf[:, lo:hi],
                                     AF.Derivative_Erf, bias=zbias, scale=1.0)

            out3 = out_d.rearrange("p (P hd) -> P p hd", P=JT // 2)

            def emit_jtile(t, ps):
                out = ps[:, (t % 2) * D : (t % 2 + 1) * D]
                for k in range(2):
                    ls = t + k
                    nc.tensor.matmul(
                        out,
                        wbuf[:, ls * WWIN + (1 - k) * 128 : ls * WWIN + (2 - k) * 128],
                        vbuf[:, ls * D : (ls + 1) * D],
                        start=(k == 0),
                        stop=(k == 1),
                    )

            def emit_pair_copy(pair, ps, ob):
                if COPY_ENG[pair] == "a":
                    nc.scalar.activation(ob[:], ps[:], AF.Copy)
                else:
                    nc.vector.tensor_copy(ob[:], ps[:])

            # direct W slabs 0,1 on ACT; z' precompute runs ahead on
            # DVE/GpSimd (only the cst DMA gates either)
            for s in ACT_DIRECT:
                emit_w_direct(s)
            for s in DVE_TS:
                emit_ts(s)
            for s in GPS_TS:
                emit_ts(s)
            pairs = {}
            obs = {}
            act_pairs = []

            def finish_pair(pair):
                ps, ob = pairs[pair]
                emit_pair_copy(pair, ps, ob)
                nc.sync.dma_start(out3[pair], ob[:])

            def emit_tile(t):
                pair = t // 2
                if t % 2 == 0:
                    ps = ps0 if pair == 0 else pspool.tile(
                        [128, 2 * D], f32, tag="ps", name="ps")
                    ob = opool.tile([128, 2 * D], bf16, name="ob")
                    pairs[pair] = (ps, ob)
                emit_jtile(t, pairs[pair][0])
                if t % 2 == 1:
                    if COPY_ENG[pair] == "a":
                        act_pairs.append(pair)  # deferred past the W chain
                    else:
                        finish_pair(pair)

            emit_tile(0)   # W slabs 0,1 done directly
            next_t = 1
            for ci, (s0, ns) in enumerate(CHUNKS):
                emit_w_chunk(ci)
                # j-tile t needs W of slabs t, t+1  ->  t <= s0+ns-2
                while next_t < JT and next_t <= s0 + ns - 2:
                    emit_tile(next_t)
                    next_t += 1
            assert next_t == JT
            # optional: keep the PE array busy (HAM warm) through the DMA
            # drain so the NRT epilogue's PE clear chain runs at full clock
            for _ in range(TAILMM):
                nc.tensor.matmul(wps[:, 0:D], wscr[:], wscr2[:],
                                 start=True, stop=True)
            # ACT pair-copies after the W chain (they share the engine)
            for pair in act_pairs:
                finish_pair(pair)
            assert len(pairs) == JT // 2

    nc.compile()
    return nc


def _get_nc():
    if "nc" not in _cached:
        _cached["nc"] = build_nc()
    return _cached["nc"]


def make_in_maps(V, sigma, mu):
    """Host-side sharding: per-core padded, scaled bf16 V rows + lookup
    table ([iota | per-slab (b0h, rh)])."""
    V = np.asarray(V, dtype=np.float32)
    sigma = np.asarray(sigma, dtype=np.float32).reshape(B, N)
    mu = np.asarray(mu, dtype=np.float32).reshape(B, N)
    CW = WWIN + 4 * NSLAB + 1
    pidx = (np.arange(VROWS) % 128).astype(np.float32)
    in_maps = []
    for c in range(NCORES):
        b, h = divmod(c, 2)
        jb = h * HALF
        lo, hi = jb - 64, jb + HALF + 64
        slo, shi = max(lo, 0), min(hi, N)
        vp = np.zeros((VROWS, D), ml_dtypes.bfloat16)
        sig = np.ones(VROWS, np.float32)
        muv = np.zeros(VROWS, np.float32)
        vp[slo - lo : shi - lo] = (
            V[b, slo:shi] * np.float32(SQRT_PI_2)).astype(ml_dtypes.bfloat16)
        sig[slo - lo : shi - lo] = sigma[b, slo:shi]
        muv[slo - lo : shi - lo] = mu[b, slo:shi]
        r = (np.float32(1.0) / sig).astype(np.float32)
        q = (np.float32(-64.0) - pidx - muv).astype(np.float32)
        rh = (r * np.float32(INV_SQRT2)).astype(np.float32)
        cst = np.zeros((128, CW), np.float32)
        cst[:, 0:WWIN] = np.arange(WWIN, dtype=np.float32)[None, :]
        cst[:, WWIN : WWIN + 2 * NSLAB : 2] = q.reshape(NSLAB, 128).T
        cst[:, WWIN + 1 : WWIN + 2 * NSLAB : 2] = rh.reshape(NSLAB, 128).T
        b0h = (q * rh).astype(np.float32)
        cst[:, WWIN + 2 * NSLAB : WWIN + 4 * NSLAB : 2] = b0h.reshape(NSLAB, 128).T
        cst[:, WWIN + 2 * NSLAB + 1 : WWIN + 4 * NSLAB : 2] = rh.reshape(NSLAB, 128).T
        # cst[:, CW-1] stays zero (bias column for the chunked D_ERF)
        vp2 = np.ascontiguousarray(
            vp.reshape(NSLAB, 128, D).transpose(1, 0, 2).reshape(128, NSLAB * D))
        iota_bf = np.arange(WWIN, dtype=np.float32).astype(ml_dtypes.bfloat16)
        cst2 = np.broadcast_to(iota_bf, (128, WWIN)).copy()
        in_maps.append({"Vp": vp2, "cst": cst, "cst2": cst2})
    return in_maps


def gather(results):
    out = np.empty((B, N, D), np.float32)
    for c in range(NCORES):
        b, h = divmod(c, 2)
        o = np.asarray(results[c]["out"]).reshape(128, JT, D)
        out[b, h * HALF : (h + 1) * HALF] = (
            o.transpose(1, 0, 2).reshape(HALF, D).astype(np.float32))
    return out


def kernel(V, sigma, mu):
    nc = _get_nc()
    in_maps = make_in_maps(V, sigma, mu)
    res = run_bass_kernel_spmd(nc, in_maps, core_ids=list(range(NCORES)))
    return gather(res.results)
